# revision 8
# baseline (speedup 1.0000x reference)
"""Multi-head attention kernel for Trainium2, 8 NeuronCores.

Sharding: data-parallel on batch (B=2 -> 2 groups of 4 cores) x
tensor-parallel on heads (16 heads -> 4 heads per core).
W_q/W_k/W_v column-split (4 heads' rows each), W_o row-split; the
W_o partial sums are reduced on the host during unsharding.

Per-core kernel (all matmuls in float32r = fp32 with 11-bit mantissa,
which runs at 4x the fp32 rate on the PE):
  Q^T = (Wq_c x_q^T) + b_q          [256, 2048]   (d on partitions)
  K^T = (Wk_c x_k^T) + b_k          [256, 2048]
  V   = (x_v Wv_c^T)                [2048, 256]   (k on partitions)
  per head h, per 512-wide q-chunk, per 128-tall k-tile:
    S^T  = K_h^T(tile)^T Q_h^T      [128, 512]  (scores transposed)
    E^T  = exp(S^T / 8)             ACT engine, PSUM -> SBUF
    O^T += [V_h | 1]^T E^T          [65, 512]   row 64 = softmax denom
  O_norm^T = O^T[0:64] * (1 ones64^T . recip(denom))   (PE outer product)
  partial^T = Wo_c^T_cols O_norm^T  [1024, 2048]
b_v is folded out: softmax rows sum to 1, so V-bias passes through
attention as a constant +b_v per row; the host adds b_v @ W_o^T into b_o.
"""

import numpy as np

D_MODEL = 1024
NUM_HEADS = 16
D_K = 64
B = 2
S = 2048
N_CORES = 8
HPC = 4  # heads per core
DH = HPC * D_K  # 256: per-core slice of d_model across heads
QC = 512  # q-chunk (PSUM free dim)
NQC = S // QC
NKT = S // 128  # k-tiles of 128
NCT = D_MODEL // 128  # contraction tiles over d_model

_runner = None


def _round_f32r(x: np.ndarray) -> np.ndarray:
    """Round fp32 to 11 explicit mantissa bits (float32r), round-half-up."""
    x = np.ascontiguousarray(x, np.float32)
    b = x.view(np.uint32)
    shift = 23 - 11
    b2 = ((b + np.uint32(1 << (shift - 1))) >> shift) << shift
    return b2.astype(np.uint32).view(np.float32)


def _build_program():
    import concourse.mybir as mybir
    from concourse import bacc
    from concourse.tile import TileContext

    f32 = mybir.dt.float32
    f32r = mybir.dt.float32r
    ADD = mybir.AluOpType.add
    EXP = mybir.ActivationFunctionType.Exp

    nc = bacc.Bacc("TRN2", target_bir_lowering=False, debug=False)

    xqT = nc.declare_dram_parameter("xqT", [D_MODEL, S], f32r, isOutput=False)
    xkT = nc.declare_dram_parameter("xkT", [D_MODEL, S], f32r, isOutput=False)
    xvT = nc.declare_dram_parameter("xvT", [D_MODEL, S], f32r, isOutput=False)
    wqT = nc.declare_dram_parameter("wqT", [D_MODEL, DH], f32r, isOutput=False)
    wkT = nc.declare_dram_parameter("wkT", [D_MODEL, DH], f32r, isOutput=False)
    wvT = nc.declare_dram_parameter("wvT", [D_MODEL, DH], f32r, isOutput=False)
    woT = nc.declare_dram_parameter("woT", [DH, D_MODEL], f32r, isOutput=False)
    bq = nc.declare_dram_parameter("bq", [DH], f32, isOutput=False)
    bk = nc.declare_dram_parameter("bk", [DH], f32, isOutput=False)
    out = nc.declare_dram_parameter("out", [D_MODEL, S], f32, isOutput=True)

    with TileContext(nc) as tc:
        with tc.tile_pool(name="persist", bufs=1) as persist:
            wq_sb = persist.tile([128, NCT, DH], f32r)
            wk_sb = persist.tile([128, NCT, DH], f32r)
            wv_sb = persist.tile([128, NCT, DH], f32r)
            wo_sb = persist.tile([128, 2, D_MODEL], f32r)
            bq_sb = persist.tile([128, 2], f32)
            bk_sb = persist.tile([128, 2], f32)
            qT_sb = persist.tile([128, 2, S], f32r)
            kT_sb = persist.tile([128, 2, S], f32r)
            vaug = persist.tile([128, NKT, HPC, D_K + 1], f32r)
            ones64 = persist.tile([1, 64], f32r)
            ones_f32 = persist.tile([128, 64], f32)

            nc.sync.dma_start(wq_sb[:], wqT.rearrange("(c p) d -> p c d", p=128))
            nc.sync.dma_start(wk_sb[:], wkT.rearrange("(c p) d -> p c d", p=128))
            nc.sync.dma_start(wv_sb[:], wvT.rearrange("(c p) d -> p c d", p=128))
            nc.sync.dma_start(wo_sb[:], woT.rearrange("(t p) m -> p t m", p=128))
            nc.sync.dma_start(bq_sb[:], bq.rearrange("(t p) -> p t", p=128))
            nc.sync.dma_start(bk_sb[:], bk.rearrange("(t p) -> p t", p=128))
            nc.vector.memset(ones_f32[:], 1.0)
            nc.vector.tensor_copy(ones64[:], ones_f32[0:1, :])
            nc.vector.tensor_copy(
                vaug[:, :, :, D_K : D_K + 1],
                ones_f32[:, :].rearrange("p (a b c) -> p a b c", a=NKT, b=HPC),
            )

            # ---- Phase A: projections (K, V first -- attention needs them
            # in full; Q streams per q-chunk) ----
            with tc.tile_pool(name="xin", bufs=6) as xin_pool, \
                 tc.tile_pool(name="psA", bufs=4, space="PSUM") as psA:
                for x_dram, w_sb, b_sb, dst in (
                    (xkT, wk_sb, bk_sb, kT_sb),
                    (xqT, wq_sb, bq_sb, qT_sb),
                ):
                    for qc in range(NQC):
                        acc = [psA.tile([128, QC], f32, tag="pa", name=f"accqk{qc}_{i}") for i in range(2)]
                        for ct in range(NCT):
                            xt = xin_pool.tile([128, QC], f32r, tag="xin", name="xt")
                            nc.sync.dma_start(
                                xt[:],
                                x_dram[ct * 128 : (ct + 1) * 128, qc * QC : (qc + 1) * QC],
                            )
                            for dt in range(2):
                                nc.tensor.matmul(
                                    acc[dt][:],
                                    w_sb[:, ct, dt * 128 : (dt + 1) * 128],
                                    xt[:],
                                    start=(ct == 0),
                                    stop=(ct == NCT - 1),
                                )
                        for dt in range(2):
                            nc.vector.tensor_scalar(
                                out=dst[:, dt, qc * QC : (qc + 1) * QC],
                                in0=acc[dt][:],
                                scalar1=b_sb[:, dt : dt + 1],
                                scalar2=None,
                                op0=ADD,
                            )
                # V projection: V[k, d] via lhsT = x_v^T tiles (stationary)
                for kg in range(4):  # 4 groups of 4 k-tiles (512 k each)
                    acc = [psA.tile([128, DH], f32, tag="pa", name=f"accv{kg}_{i}") for i in range(4)]
                    for ct in range(NCT):
                        xt = xin_pool.tile([128, QC], f32r, tag="xin", name="xt")
                        nc.sync.dma_start(
                            xt[:],
                            xvT[ct * 128 : (ct + 1) * 128, kg * QC : (kg + 1) * QC],
                        )
                        for j in range(4):
                            nc.tensor.matmul(
                                acc[j][:],
                                xt[:, j * 128 : (j + 1) * 128],
                                wv_sb[:, ct, :],
                                start=(ct == 0),
                                stop=(ct == NCT - 1),
                            )
                    for j in range(4):
                        nc.vector.tensor_copy(
                            vaug[:, kg * 4 + j, :, 0:D_K],
                            acc[j][:].rearrange("p (h d) -> p h d", h=HPC),
                        )

            # ---- Phase B: attention ----
            otn_pool = tc.alloc_tile_pool(name="otn", bufs=8)
            otn_tiles = {}
            with tc.tile_pool(name="psQK", bufs=4, space="PSUM") as psQK, \
                 tc.tile_pool(name="psPV", bufs=2, space="PSUM") as psPV, \
                 tc.tile_pool(name="psBC", bufs=2, space="PSUM") as psBC, \
                 tc.tile_pool(name="epool", bufs=4) as epool, \
                 tc.tile_pool(name="rbc", bufs=4) as rbc_pool:
                for qc in range(NQC):
                    qsl = slice(qc * QC, (qc + 1) * QC)
                    for dt in range(2):  # head pair (2*dt, 2*dt+1)
                        po = [psPV.tile([D_K + 1, QC], f32, tag="po", name=f"po{qc}_{dt}_{i}") for i in range(2)]
                        for kt in range(NKT):
                            ss = []
                            for j in range(2):
                                s = psQK.tile([128, QC], f32, tag="s", name=f"s{qc}_{dt}_{kt}_{j}")
                                nc.tensor.matmul(
                                    s[:],
                                    kT_sb[j * 64 : (j + 1) * 64, dt, kt * 128 : (kt + 1) * 128],
                                    qT_sb[j * 64 : (j + 1) * 64, dt, qsl],
                                    start=True,
                                    stop=True,
                                )
                                ss.append(s)
                            for j in range(2):
                                e = epool.tile([128, QC], f32r, tag="e", name=f"e{qc}_{dt}_{kt}_{j}")
                                nc.scalar.activation(e[:], ss[j][:], EXP, scale=0.125)
                                nc.tensor.matmul(
                                    po[j][:],
                                    vaug[:, kt, 2 * dt + j, :],
                                    e[:],
                                    start=(kt == 0),
                                    stop=(kt == NKT - 1),
                                )
                        ot = otn_pool.tile([128, QC], f32r, tag="ot", name=f"ot{qc}_{dt}")
                        for j in range(2):
                            rc = rbc_pool.tile([1, QC], f32r, tag="rc", name=f"rc{qc}_{dt}_{j}")
                            with nc.allow_low_precision("softmax denom recip"):
                                nc.vector.reciprocal(rc[:], po[j][D_K : D_K + 1, :])
                            bc = psBC.tile([64, QC], f32, tag="bc", name=f"bc{qc}_{dt}_{j}")
                            nc.tensor.matmul(bc[:], ones64[:], rc[:], start=True, stop=True)
                            bcs = rbc_pool.tile([64, QC], f32, tag="bcs", name=f"bcs{qc}_{dt}_{j}")
                            nc.vector.tensor_copy(bcs[:], bc[:])
                            nc.vector.tensor_mul(
                                ot[j * 64 : (j + 1) * 64, :], po[j][0:D_K, :], bcs[:]
                            )
                        otn_tiles[(qc, dt)] = ot

            # ---- Phase C: output projection (partial^T) ----
            with tc.tile_pool(name="psO", bufs=4, space="PSUM") as psO, \
                 tc.tile_pool(name="osb", bufs=4) as osb:
                for qc in range(NQC):
                    qsl = slice(qc * QC, (qc + 1) * QC)
                    for mt in range(8):
                        p = psO.tile([128, QC], f32, tag="p", name=f"p{qc}_{mt}")
                        for dt in range(2):
                            nc.tensor.matmul(
                                p[:],
                                wo_sb[:, dt, mt * 128 : (mt + 1) * 128],
                                otn_tiles[(qc, dt)][:],
                                start=(dt == 0),
                                stop=(dt == 1),
                            )
                        ob = osb.tile([128, QC], f32, tag="ob", name=f"ob{qc}_{mt}")
                        nc.vector.tensor_copy(ob[:], p[:])
                        nc.sync.dma_start(out[mt * 128 : (mt + 1) * 128, qsl], ob[:])
            otn_pool.release()

    nc.finalize()
    return nc


class _Runner:
    """Compiles the SPMD program once and keeps the jitted callable."""

    IN_NAMES = ["xqT", "xkT", "xvT", "wqT", "wkT", "wvT", "woT", "bq", "bk"]
    OUT_NAME = "out"

    def __init__(self):
        import jax
        import jax.numpy as jnp
        from jax.sharding import Mesh, PartitionSpec
        try:
            from jax.experimental.shard_map import shard_map
        except ImportError:
            from jax.shard_map import shard_map
        from concourse import bass2jax
        import concourse.mybir as mybir

        self.jax = jax
        nc = _build_program()
        bass2jax.install_neuronx_cc_hook()

        partition_name = (
            nc.partition_id_tensor.name if nc.partition_id_tensor else None
        )
        in_names = []
        out_names = []
        out_avals = []
        for alloc in nc.m.functions[0].allocations:
            if not isinstance(alloc, mybir.MemoryLocationSet):
                continue
            name = alloc.memorylocations[0].name
            if alloc.kind == "ExternalInput":
                if name != partition_name:
                    in_names.append(name)
            elif alloc.kind == "ExternalOutput":
                out_names.append(name)
                out_avals.append(
                    jax.core.ShapedArray(
                        tuple(alloc.tensor_shape), mybir.dt.np(alloc.dtype)
                    )
                )
        self.in_names = in_names
        self.out_names = out_names
        self.out_avals = out_avals
        n_params = len(in_names)

        all_in_names = in_names + out_names
        if partition_name is not None:
            all_in_names = all_in_names + [partition_name]

        def _body(*args):
            operands = list(args)
            if partition_name is not None:
                operands.append(bass2jax.partition_id_tensor())
            outs = bass2jax._bass_exec_p.bind(
                *operands,
                out_avals=tuple(out_avals),
                in_names=tuple(all_in_names),
                out_names=tuple(out_names),
                lowering_input_output_aliases=(),
                sim_require_finite=True,
                sim_require_nnan=True,
                nc=nc,
            )
            return tuple(outs)

        devices = jax.devices()[:N_CORES]
        mesh = Mesh(np.asarray(devices), ("core",))
        in_specs = (PartitionSpec("core"),) * (n_params + len(out_names))
        out_specs = (PartitionSpec("core"),) * len(out_names)
        self._sharded = jax.jit(
            shard_map(
                _body, mesh=mesh, in_specs=in_specs, out_specs=out_specs,
                check_rep=False,
            ),
            keep_unused=True,
        )
        self._mesh = mesh
        # device-resident dummy buffers for the NEFF output bindings
        self._zeros = [
            jnp.zeros((N_CORES * a.shape[0], *a.shape[1:]), a.dtype)
            for a in out_avals
        ]
        self._jnp = jnp

    def put_inputs(self, in_maps):
        """Concatenate per-core inputs along axis 0 (device-shardable)."""
        concat = [
            np.concatenate([np.asarray(in_maps[c][n]) for c in range(N_CORES)], axis=0)
            for n in self.in_names
        ]
        return [self._jnp.asarray(a) for a in concat]

    def run(self, dev_inputs):
        outs = self._sharded(*dev_inputs, *self._zeros)
        return [o.block_until_ready() for o in outs]

    def split_outputs(self, outs):
        """-> list per core of {name: np.ndarray}"""
        res = []
        for c in range(N_CORES):
            d = {}
            for i, name in enumerate(self.out_names):
                a = self.out_avals[i]
                d[name] = np.asarray(outs[i]).reshape(N_CORES, *a.shape)[c]
            res.append(d)
        return res


def _get_runner():
    global _runner
    if _runner is None:
        _runner = _Runner()
    return _runner


def prepare_in_maps(query, key, value, W_q, b_q, W_k, b_k, W_v, b_v, W_o, b_o):
    query = np.asarray(query, np.float32)
    key = np.asarray(key, np.float32)
    value = np.asarray(value, np.float32)
    in_maps = []
    for c in range(N_CORES):
        b = c // 4
        g = c % 4
        rows = slice(g * DH, (g + 1) * DH)
        in_maps.append({
            "xqT": _round_f32r(query[b].T),
            "xkT": _round_f32r(key[b].T),
            "xvT": _round_f32r(value[b].T),
            "wqT": _round_f32r(np.asarray(W_q)[rows].T),
            "wkT": _round_f32r(np.asarray(W_k)[rows].T),
            "wvT": _round_f32r(np.asarray(W_v)[rows].T),
            "woT": _round_f32r(np.asarray(W_o)[:, rows].T),
            "bq": np.ascontiguousarray(np.asarray(b_q)[rows], np.float32),
            "bk": np.ascontiguousarray(np.asarray(b_k)[rows], np.float32),
        })
    return in_maps


def postprocess(core_outs, b_v, W_o, b_o):
    """Sum the 4 tensor-parallel partials per batch, add the folded bias."""
    bo_eff = (
        np.asarray(b_o, np.float64)
        + np.asarray(b_v, np.float64) @ np.asarray(W_o, np.float64).T
    )
    out = np.empty((B, S, D_MODEL), np.float32)
    for b in range(B):
        acc = np.zeros((D_MODEL, S), np.float64)
        for g in range(4):
            acc += core_outs[4 * b + g]["out"]
        out[b] = (acc.T + bo_eff).astype(np.float32)
    return out


def kernel(**inputs) -> np.ndarray:
    r = _get_runner()
    in_maps = prepare_in_maps(**inputs)
    dev_in = r.put_inputs(in_maps)
    outs = r.run(dev_in)
    core_outs = r.split_outputs(outs)
    return postprocess(core_outs, inputs["b_v"], inputs["W_o"], inputs["b_o"])
